# revision 1
# baseline (speedup 1.0000x reference)
"""Trainium2 Bass kernel for the GRU caption model.

Computes: h0 = feat @ W_hp.T + b_hp; 200-step GRU with constant hidden-proj
gate pre-activations; logits = outs @ W_out.T + b_out -> [B, V, T].

Strategy: every core runs the (tiny, latency-bound) GRU redundantly; the
vocab dimension of W_out is sharded 8 ways; each core emits its own
[B, 3840, T] logits slice which the host concatenates.

All on-chip compute uses a transposed [feature-on-partitions, batch-free]
layout so the recurrent state feeds the next step's matmul directly.
"""

import numpy as np
import ml_dtypes

import concourse.bass as bass
import concourse.mybir as mybir
import concourse.tile as tile
from concourse import bacc
from concourse.bass_utils import run_bass_kernel_spmd

F32 = mybir.dt.float32
F32R = mybir.dt.float32r
BF16 = mybir.dt.bfloat16
AF = mybir.ActivationFunctionType
ALU = mybir.AluOpType

VOCAB = 30522
HID = 512
FEAT = 2048
STEPS = 200
BATCH = 32
SOS = 101
NCORES = 8
P = 128
KO = HID // P          # 4 h-chunks
GM = 3 * HID // P      # 12 gate row-groups (r: 0-3, z: 4-7, n: 8-11)
KF = FEAT // P         # 16 feat chunks
VPAD = 3840            # per-core padded vocab rows = 30 * 128
MT = VPAD // P         # 30 vocab tiles per core
TBLOCKS = [(0, 64), (64, 128), (128, 200)]  # proj t-blocks

LAST_RESULTS = None  # test harness introspection
EMIT_GRU = True    # variant switch (sim experiments)
EMIT_PROJ = True   # variant switch (sim experiments)
PROJ_MODE = 2      # 0 = matmuls only, 1 = +copies, 2 = +DMA (sim experiments)


def _r(ap):
    """Reinterpret an fp32 AP as float32r for full-rate PE streaming."""
    return ap.bitcast(F32R)


def build():
    nc = bacc.Bacc("TRN2", target_bir_lowering=False, debug=False)

    featT = nc.dram_tensor("featT", [FEAT, BATCH], F32, kind="ExternalInput")
    WhpT = nc.dram_tensor("WhpT", [FEAT, HID], F32, kind="ExternalInput")
    WihT = nc.dram_tensor("WihT", [HID, 3 * HID], BF16, kind="ExternalInput")
    WhhT = nc.dram_tensor("WhhT", [HID, 3 * HID], F32, kind="ExternalInput")
    b_ih = nc.dram_tensor("b_ih", [3 * HID], F32, kind="ExternalInput")
    b_hh = nc.dram_tensor("b_hh", [3 * HID], F32, kind="ExternalInput")
    b_hp = nc.dram_tensor("b_hp", [HID], F32, kind="ExternalInput")
    x0T = nc.dram_tensor("x0T", [HID, BATCH], BF16, kind="ExternalInput")
    WoutT = nc.dram_tensor("WoutT", [HID, VPAD], F32R, kind="ExternalInput")
    b_out = nc.dram_tensor("b_out", [VPAD], F32, kind="ExternalInput")
    OUT = nc.dram_tensor("OUT", [BATCH, VPAD, STEPS], F32, kind="ExternalOutput")

    with tile.TileContext(nc) as tc:
        with (
            tc.tile_pool(name="const", bufs=1) as const,
            tc.tile_pool(name="stream", bufs=3) as stream,
            tc.tile_pool(name="step", bufs=4) as sp,
            tc.tile_pool(name="hb", bufs=4) as hb,
            tc.tile_pool(name="outp", bufs=6) as outp,
            tc.tile_pool(name="psg", bufs=3, space="PSUM") as psg,
            tc.tile_pool(name="psp", bufs=4, space="PSUM") as psp,
        ):
            # ---- constants into SBUF ----
            wih = const.tile([P, KO, GM, P], BF16, tag="wih")
            nc.sync.dma_start(
                wih[:], WihT.rearrange("(k p) (m c) -> p k m c", p=P, c=P)
            )
            featT_sb = const.tile([P, KF, BATCH], F32, tag="featsb")
            nc.sync.dma_start(featT_sb[:], featT.rearrange("(k p) b -> p k b", p=P))
            bih_sb = const.tile([P, GM], F32, tag="bih")
            nc.sync.dma_start(bih_sb[:], b_ih.rearrange("(m p) -> p m", p=P))
            bhh_sb = const.tile([P, GM], F32, tag="bhh")
            nc.sync.dma_start(bhh_sb[:], b_hh.rearrange("(m p) -> p m", p=P))
            bhp_sb = const.tile([P, KO], F32, tag="bhp")
            nc.sync.dma_start(bhp_sb[:], b_hp.rearrange("(m p) -> p m", p=P))
            bout_sb = const.tile([P, MT], F32, tag="bout")
            nc.sync.dma_start(bout_sb[:], b_out.rearrange("(m p) -> p m", p=P))

            WhpT_r = WhpT.rearrange("(k p) h -> p k h", p=P)
            WhhT_r = WhhT.rearrange("(k p) g -> p k g", p=P)
            WoutT_r = WoutT.rearrange("(k p) v -> p k v", p=P)

            # ---- h0 = feat @ W_hp.T + b_hp (fp32, exact) ----
            ps_h = psg.tile([P, GM, BATCH], F32, tag="gates")
            for ko in range(KO):
                for kf in range(KF):
                    wt = stream.tile([P, P], F32, tag="whp")
                    nc.sync.dma_start(wt[:], WhpT_r[:, kf, ko * P:(ko + 1) * P])
                    nc.tensor.matmul(
                        ps_h[:, ko, :], wt[:], featT_sb[:, kf, :],
                        start=(kf == 0), stop=(kf == KF - 1),
                    )
            h0T = const.tile([P, KO, BATCH], F32, tag="h0T")
            for ko in range(KO):
                nc.scalar.activation(
                    h0T[:, ko, :], ps_h[:, ko, :], AF.Identity,
                    bias=bhp_sb[:, ko, None], scale=1.0,
                )
            h0_half = const.tile([P, KO, BATCH], F32, tag="h0h")
            nc.scalar.mul(h0_half[:], h0T[:], 0.5)

            # ---- gh = h0 @ W_hh.T + b_hh (fp32, exact; step-invariant) ----
            ps_g = psg.tile([P, GM, BATCH], F32, tag="gates")
            for m in range(GM):
                for k in range(KO):
                    wt = stream.tile([P, P], F32, tag="whh")
                    nc.sync.dma_start(wt[:], WhhT_r[:, k, m * P:(m + 1) * P])
                    nc.tensor.matmul(
                        ps_g[:, m, :], wt[:], h0T[:, k, :],
                        start=(k == 0), stop=(k == KO - 1),
                    )
            ghT = const.tile([P, GM, BATCH], F32, tag="ghT")
            for m in range(GM):
                nc.scalar.activation(
                    ghT[:, m, :], ps_g[:, m, :], AF.Identity,
                    bias=bhh_sb[:, m, None], scale=1.0,
                )
            # C_rz = gh_rz + b_ih_rz ; hn2 = 0.5*gh_n ; E_n = hn2 + b_ih_n
            C_rz = const.tile([P, 8, BATCH], F32, tag="Crz")
            nc.vector.tensor_add(
                C_rz[:], ghT[:, 0:8, :],
                bih_sb[:, 0:8, None].to_broadcast((P, 8, BATCH)),
            )
            hn2 = const.tile([P, KO, BATCH], F32, tag="hn2")
            nc.scalar.mul(hn2[:], ghT[:, 8:12, :], 0.5)
            E_n = const.tile([P, KO, BATCH], F32, tag="En")
            nc.vector.tensor_add(
                E_n[:], hn2[:],
                bih_sb[:, 8:12, None].to_broadcast((P, KO, BATCH)),
            )

            # resT blocks: col = b*bsize + (t - t0), per h-chunk ko
            resT = []
            for j, (t0, t1) in enumerate(TBLOCKS):
                bs = t1 - t0
                rt = const.tile(
                    [P, KO, BATCH, bs], F32R, tag=f"resT{j}", name=f"resT{j}"
                )
                resT.append(rt)

            prev = hb.tile([P, KO, BATCH], BF16, tag="hb")
            nc.sync.dma_start(prev[:], x0T.rearrange("(k p) b -> p k b", p=P))

            def proj_block(j):
                t0, t1 = TBLOCKS[j]
                bs = t1 - t0
                gb = 4
                N = gb * bs
                for m in range(MT):
                    wt = stream.tile([P, KO, P], F32R, tag="wout")
                    nc.sync.dma_start(wt[:], WoutT_r[:, :, m * P:(m + 1) * P])
                    for g in range(BATCH // gb):
                        ps_full = psp.tile([P, 288], F32, tag="pp", name="pp")
                        ps = ps_full[:, :N]
                        for k in range(KO):
                            nc.tensor.matmul(
                                ps,
                                wt[:, k, :],
                                resT[j][:, k, gb * g:gb * g + gb, :],
                                start=(k == 0), stop=(k == KO - 1),
                            )
                        if PROJ_MODE == 0:
                            continue
                        ob_full = outp.tile([P, 288], F32, tag="ob", name="ob")
                        ob = ob_full[:, :N]
                        if (m + g) % 2 == 0:
                            nc.scalar.activation(
                                ob, ps, AF.Identity,
                                bias=bout_sb[:, m, None], scale=1.0,
                            )
                        else:
                            nc.vector.tensor_scalar_add(ob, ps, bout_sb[:, m, None])
                        if PROJ_MODE >= 2:
                            dst = OUT[
                                gb * g:gb * g + gb, m * P:(m + 1) * P, t0:t1
                            ].rearrange("b v t -> v b t")
                            nc.sync.dma_start(
                                dst, ob.rearrange("p (b t) -> p b t", b=gb)
                            )

            # ---- GRU steps ----
            if not EMIT_GRU:
                for j in range(len(TBLOCKS)):
                    nc.vector.memset(resT[j][:], 0.25)
                    proj_block(j)
            mm_order = [8, 9, 10, 11] + list(range(8))  # n-gates first
            for t in range(STEPS if EMIT_GRU else 0):
                ps = psg.tile([P, GM, BATCH], F32, tag="gates")
                for m in mm_order:
                    for k in range(KO):
                        nc.tensor.matmul(
                            ps[:, m, :], wih[:, k, m, :], prev[:, k, :],
                            start=(k == 0), stop=(k == KO - 1),
                        )
                s_rz = sp.tile([P, 8, BATCH], F32, tag="srz")
                nc.vector.tensor_add(s_rz[:], ps[:, 0:8, :], C_rz[:])
                t_rz = sp.tile([P, 8, BATCH], F32, tag="trz")
                nc.scalar.activation(t_rz[:], s_rz[:], AF.Tanh, scale=0.5)
                a = sp.tile([P, KO, BATCH], F32, tag="a")
                nc.vector.tensor_mul(a[:], t_rz[:, 0:4, :], hn2[:])
                sn1 = sp.tile([P, KO, BATCH], F32, tag="sn1")
                nc.vector.tensor_add(sn1[:], ps[:, 8:12, :], E_n[:])
                sn2 = sp.tile([P, KO, BATCH], F32, tag="sn2")
                nc.vector.tensor_add(sn2[:], sn1[:], a[:])
                n = sp.tile([P, KO, BATCH], F32, tag="n")
                nc.scalar.activation(n[:], sn2[:], AF.Tanh, scale=1.0)
                q = sp.tile([P, KO, BATCH], F32, tag="q")
                nc.vector.tensor_sub(q[:], h0T[:], n[:])
                w2 = sp.tile([P, KO, BATCH], F32, tag="w2")
                nc.vector.scalar_tensor_tensor(
                    w2[:], t_rz[:, 4:8, :], 0.5, q[:], ALU.mult, ALU.mult
                )
                p2 = sp.tile([P, KO, BATCH], F32, tag="p2")
                nc.vector.scalar_tensor_tensor(
                    p2[:], n[:], 0.5, h0_half[:], ALU.mult, ALU.add
                )
                nxt = hb.tile([P, KO, BATCH], BF16, tag="hb")
                nc.vector.tensor_add(nxt[:], w2[:], p2[:])
                j = next(i for i, (a, b) in enumerate(TBLOCKS) if a <= t < b)
                t0 = TBLOCKS[j][0]
                nc.gpsimd.tensor_add(resT[j][:, :, :, t - t0], w2[:], p2[:])
                prev = nxt
                if t == TBLOCKS[j][1] - 1 and EMIT_PROJ:
                    proj_block(j)

    nc.compile()
    return nc


def _shard_inputs(feat, W_hp, b_hp, W_ih, W_hh, b_ih, b_hh, embed, W_out, b_out):
    bf = ml_dtypes.bfloat16
    featT = np.ascontiguousarray(feat.T, dtype=np.float32)
    WhpT = np.ascontiguousarray(W_hp.T, dtype=np.float32)
    WihT = np.ascontiguousarray(W_ih.T).astype(bf)
    WhhT = np.ascontiguousarray(W_hh.T, dtype=np.float32)
    x0T = np.ascontiguousarray(
        np.repeat(np.asarray(embed)[SOS][:, None], BATCH, axis=1)
    ).astype(bf)
    Wo = np.zeros((NCORES * VPAD, HID), np.float32)
    Wo[:VOCAB] = W_out
    bo = np.zeros((NCORES * VPAD,), np.float32)
    bo[:VOCAB] = b_out
    common = dict(
        featT=featT, WhpT=WhpT, WihT=WihT, WhhT=WhhT,
        b_ih=np.asarray(b_ih, np.float32), b_hh=np.asarray(b_hh, np.float32),
        b_hp=np.asarray(b_hp, np.float32), x0T=x0T,
    )
    in_maps = []
    for c in range(NCORES):
        sl = slice(c * VPAD, (c + 1) * VPAD)
        m = dict(common)
        m["WoutT"] = np.ascontiguousarray(Wo[sl].T)
        m["b_out"] = bo[sl].copy()
        in_maps.append(m)
    return in_maps


def kernel(**inputs):
    global LAST_RESULTS
    args = {k: np.asarray(v) for k, v in inputs.items()}
    in_maps = _shard_inputs(
        args["feat"], args["W_hp"], args["b_hp"], args["W_ih"], args["W_hh"],
        args["b_ih"], args["b_hh"], args["embed"], args["W_out"], args["b_out"],
    )
    nc = build()
    res = run_bass_kernel_spmd(nc, in_maps, core_ids=list(range(NCORES)))
    LAST_RESULTS = res
    out = np.concatenate([r["OUT"] for r in res.results], axis=1)[:, :VOCAB, :]
    return np.ascontiguousarray(out, dtype=np.float32)



# revision 4
# speedup vs baseline: 1.4674x; 1.4674x over previous
"""Trainium2 Bass kernel for the GRU caption model.

h0 = feat @ W_hp.T + b_hp; 200-step GRU whose hidden-proj gate pre-activations
are step-invariant; logits = h_t @ W_out.T + b_out -> [B, V, T].

v2 design (CoreSim cost-model driven):
- Vocab sharded 8 ways; every core runs the (latency-bound) GRU redundantly.
- GRU per-step chain minimized: step-invariant gate constants are preloaded
  into PSUM with a single N=384 fp32r identity matmul, the elementwise chain
  runs on Pool (GPSIMD, no SBUF/PSUM access-latency penalty) with the two
  tanhs on Act.  h_t is written once, in bf16, straight into the resT
  activation buffer that feeds both the next step's matmul and the
  projection.
- Projection in 4 t-blocks of 50 steps, bf16 weights/activations (1 cyc/row
  at any N), bias folded into the PSUM->SBUF copy (split between DVE+Pool).
- Output DMA: one transfer per (vocab-tile, t-block) with 6400-byte
  contiguous runs (full 360 GB/s); per-block DRAM tensors laid out
  [VPAD, B, BS], transposed/stitched on the host.
"""

import numpy as np
import ml_dtypes

import concourse.bass as bass
import concourse.mybir as mybir
import concourse.tile as tile
from concourse import bacc
from concourse.bass_utils import run_bass_kernel_spmd

F32 = mybir.dt.float32
F32R = mybir.dt.float32r
BF16 = mybir.dt.bfloat16
AF = mybir.ActivationFunctionType
ALU = mybir.AluOpType

VOCAB = 30522
HID = 512
FEAT = 2048
STEPS = 200
BATCH = 32
SOS = 101
NCORES = 8
P = 128
KO = HID // P          # 4 h-chunks
GM = 3 * HID // P      # 12 gate row-groups (r: 0-3, z: 4-7, n: 8-11)
KF = FEAT // P         # 16 feat chunks
VPAD = 3840            # per-core padded vocab rows = 30 * 128
MT = VPAD // P         # 30 vocab tiles per core
NBLK = 4
BS = STEPS // NBLK     # 50 steps per proj block
PBG = 4                # batch group size in proj (N = PBG*BS = 200)

LAST_RESULTS = None  # test harness introspection


def build():
    nc = bacc.Bacc("TRN2", target_bir_lowering=False, debug=False)

    featT = nc.dram_tensor("featT", [FEAT, BATCH], F32, kind="ExternalInput")
    WhpT = nc.dram_tensor("WhpT", [FEAT, HID], F32, kind="ExternalInput")
    WihT = nc.dram_tensor("WihT", [HID, 3 * HID], BF16, kind="ExternalInput")
    WhhT = nc.dram_tensor("WhhT", [HID, 3 * HID], F32, kind="ExternalInput")
    b_ih = nc.dram_tensor("b_ih", [3 * HID], F32, kind="ExternalInput")
    b_hh = nc.dram_tensor("b_hh", [3 * HID], F32, kind="ExternalInput")
    b_hp = nc.dram_tensor("b_hp", [HID], F32, kind="ExternalInput")
    x0T = nc.dram_tensor("x0T", [HID, BATCH], BF16, kind="ExternalInput")
    ident = nc.dram_tensor("ident", [P, P], F32, kind="ExternalInput")
    WoutT = nc.dram_tensor("WoutT", [HID, VPAD], BF16, kind="ExternalInput")
    b_out = nc.dram_tensor("b_out", [VPAD], F32, kind="ExternalInput")
    OUTS = [
        nc.dram_tensor(f"OUT{j}", [VPAD, BATCH, BS], F32, kind="ExternalOutput")
        for j in range(NBLK)
    ]

    with tile.TileContext(nc) as tc:
        with (
            tc.tile_pool(name="const", bufs=1) as const,
            tc.tile_pool(name="sp", bufs=4) as sp,
            tc.tile_pool(name="stg", bufs=3) as stg,
            tc.tile_pool(name="psg", bufs=2, space="PSUM") as psg,
            tc.tile_pool(name="psp", bufs=4, space="PSUM") as psp,
        ):
            # ---- constants into SBUF (few, large DMAs) ----
            featT_sb = const.tile([P, KF, BATCH], F32, tag="featsb")
            nc.sync.dma_start(featT_sb[:], featT.rearrange("(k p) b -> p k b", p=P))
            whp_sb = const.tile([P, KF, HID], F32, tag="whp")
            nc.sync.dma_start(whp_sb[:], WhpT.rearrange("(k p) h -> p k h", p=P))
            bhp_sb = const.tile([P, KO], F32, tag="bhp")
            nc.sync.dma_start(bhp_sb[:], b_hp.rearrange("(m p) -> p m", p=P))
            bih_sb = const.tile([P, GM], F32, tag="bih")
            nc.sync.dma_start(bih_sb[:], b_ih.rearrange("(m p) -> p m", p=P))
            bhh_sb = const.tile([P, GM], F32, tag="bhh")
            nc.sync.dma_start(bhh_sb[:], b_hh.rearrange("(m p) -> p m", p=P))
            id_sb = const.tile([P, P], F32, tag="id")
            nc.sync.dma_start(id_sb[:], ident[:, :])
            x0_sb = const.tile([P, KO, BATCH], BF16, tag="x0")
            nc.sync.dma_start(x0_sb[:], x0T.rearrange("(k p) b -> p k b", p=P))
            whh_sb = const.tile([P, KO, 3 * HID], F32, tag="whh")
            nc.sync.dma_start(whh_sb[:], WhhT.rearrange("(k p) g -> p k g", p=P))
            wih = const.tile([P, KO, GM, P], BF16, tag="wih")
            nc.sync.dma_start(
                wih[:], WihT.rearrange("(k p) (m c) -> p k m c", p=P, c=P)
            )
            wout_sb = const.tile([P, KO, VPAD], BF16, tag="wout")
            nc.sync.dma_start(
                wout_sb[:], WoutT.rearrange("(k p) v -> p k v", p=P)
            )
            bout_sb = const.tile([P, MT], F32, tag="bout")
            nc.sync.dma_start(bout_sb[:], b_out.rearrange("(m p) -> p m", p=P))

            # ---- h0 = feat @ W_hp.T + b_hp (fp32, exact) ----
            ps_h = psg.tile([P, GM, BATCH], F32, tag="gates", name="psh")
            for ko in range(KO):
                for kf in range(KF):
                    nc.tensor.matmul(
                        ps_h[:, ko, :],
                        whp_sb[:, kf, ko * P:(ko + 1) * P],
                        featT_sb[:, kf, :],
                        start=(kf == 0), stop=(kf == KF - 1),
                    )
            h0T = const.tile([P, KO, BATCH], F32, tag="h0T")
            for ko in range(KO):
                nc.scalar.activation(
                    h0T[:, ko, :], ps_h[:, ko, :], AF.Identity,
                    bias=bhp_sb[:, ko, None], scale=1.0,
                )
            h0_half = const.tile([P, KO, BATCH], F32, tag="h0h")
            nc.scalar.mul(h0_half[:], h0T[:], 0.5)

            # ---- gh = h0 @ W_hh.T + b_hh (fp32, exact; step-invariant) ----
            ps_g = psg.tile([P, GM, BATCH], F32, tag="gates", name="psg2")
            for m in range(GM):
                for k in range(KO):
                    nc.tensor.matmul(
                        ps_g[:, m, :],
                        whh_sb[:, k, m * P:(m + 1) * P],
                        h0T[:, k, :],
                        start=(k == 0), stop=(k == KO - 1),
                    )
            ghT = const.tile([P, GM, BATCH], F32, tag="ghT")
            for m in range(GM):
                nc.scalar.activation(
                    ghT[:, m, :], ps_g[:, m, :], AF.Identity,
                    bias=bhh_sb[:, m, None], scale=1.0,
                )
            # hn2 = 0.5*gh_n (used every step by the r-gating of n)
            hn2 = const.tile([P, KO, BATCH], F32, tag="hn2")
            nc.scalar.mul(hn2[:], ghT[:, 8:12, :], 0.5)
            # C: per-step PSUM preload. rz: gh_rz + b_ih_rz ; n: hn2 + b_ih_n
            C = const.tile([P, GM, BATCH], F32, tag="C")
            nc.vector.tensor_add(
                C[:, 0:8, :], ghT[:, 0:8, :],
                bih_sb[:, 0:8, None].to_broadcast((P, 8, BATCH)),
            )
            nc.vector.tensor_add(
                C[:, 8:12, :], hn2[:],
                bih_sb[:, 8:12, None].to_broadcast((P, KO, BATCH)),
            )

            # resT blocks: h_t in bf16, feeds next-step matmul AND projection
            resT = []
            for j in range(NBLK):
                rt = const.tile(
                    [P, KO, BATCH, BS], BF16, tag=f"resT{j}", name=f"resT{j}"
                )
                resT.append(rt)

            id_r = id_sb.bitcast(F32R)
            C_r = C.bitcast(F32R)

            def proj_block(j):
                for m in range(MT):
                    stage = stg.tile([P, BATCH, BS], F32, tag="stage", name="stage")
                    for g in range(BATCH // PBG):
                        pp = psp.tile([P, PBG, BS], F32, tag="pp", name="pp")
                        for k in range(KO):
                            nc.tensor.matmul(
                                pp[:, :, :],
                                wout_sb[:, k, m * P:(m + 1) * P],
                                resT[j][:, k, PBG * g:PBG * g + PBG, :],
                                start=(k == 0), stop=(k == KO - 1),
                            )
                        dst = stage[:, PBG * g:PBG * g + PBG, :]
                        if g % 2 == 0:
                            nc.gpsimd.tensor_scalar_add(
                                dst, pp[:, :, :], bout_sb[:, m, None]
                            )
                        else:
                            nc.vector.tensor_scalar_add(
                                dst, pp[:, :, :], bout_sb[:, m, None]
                            )
                    nc.sync.dma_start(
                        OUTS[j][m * P:(m + 1) * P, :, :], stage[:]
                    )

            # ---- GRU steps ----
            for t in range(STEPS):
                j, o = divmod(t, BS)
                ps = psg.tile([P, GM, BATCH], F32, tag="gates", name="ps")
                # preload step-invariant gate constants into all 12 groups
                nc.tensor.matmul(
                    ps[:, :, :], id_r, C_r, start=True, stop=False,
                    skip_group_check=True,
                )
                if t == 0:
                    prev = x0_sb
                else:
                    jp, op = divmod(t - 1, BS)
                    prev = resT[jp][:, :, :, op]
                # gate matmuls: r groups first so the r-tanh fires earliest
                for m in range(GM):
                    for k in range(KO):
                        nc.tensor.matmul(
                            ps[:, m, :], wih[:, k, m, :], prev[:, k, :],
                            start=False, stop=(k == KO - 1),
                            skip_group_check=True,
                        )
                # r/z: tanh(0.5*(gi+gh+b)) ; r = (1+t_r)/2, z = (1+t_z)/2
                t_r = sp.tile([P, KO, BATCH], F32, tag="tr")
                nc.scalar.activation(t_r[:], ps[:, 0:4, :], AF.Tanh, scale=0.5)
                t_z = sp.tile([P, KO, BATCH], F32, tag="tz")
                nc.scalar.activation(t_z[:], ps[:, 4:8, :], AF.Tanh, scale=0.5)
                # n = tanh(gi_n + b_ih_n + r*gh_n) with r*gh_n = hn2 + t_r*hn2
                a = sp.tile([P, KO, BATCH], F32, tag="a")
                nc.gpsimd.tensor_mul(a[:], t_r[:], hn2[:])
                sn = sp.tile([P, KO, BATCH], F32, tag="sn")
                nc.gpsimd.tensor_add(sn[:], ps[:, 8:12, :], a[:])
                n = sp.tile([P, KO, BATCH], F32, tag="n")
                nc.scalar.activation(n[:], sn[:], AF.Tanh, scale=1.0)
                # h = (1-z)*n + z*h0 = [0.5n + 0.5h0] + 0.5*t_z*(h0 - n)
                q = sp.tile([P, KO, BATCH], F32, tag="q")
                nc.gpsimd.tensor_sub(q[:], h0T[:], n[:])
                w2 = sp.tile([P, KO, BATCH], F32, tag="w2")
                nc.gpsimd.scalar_tensor_tensor(
                    w2[:], t_z[:], 0.5, q[:], ALU.mult, ALU.mult
                )
                p2 = sp.tile([P, KO, BATCH], F32, tag="p2")
                nc.vector.scalar_tensor_tensor(
                    p2[:], n[:], 0.5, h0_half[:], ALU.mult, ALU.add
                )
                nc.gpsimd.tensor_add(resT[j][:, :, :, o], w2[:], p2[:])
                if o == BS - 1:
                    proj_block(j)

    nc.compile()
    return nc


def _shard_inputs(feat, W_hp, b_hp, W_ih, W_hh, b_ih, b_hh, embed, W_out, b_out):
    bf = ml_dtypes.bfloat16
    featT = np.ascontiguousarray(feat.T, dtype=np.float32)
    WhpT = np.ascontiguousarray(W_hp.T, dtype=np.float32)
    WihT = np.ascontiguousarray(W_ih.T).astype(bf)
    WhhT = np.ascontiguousarray(W_hh.T, dtype=np.float32)
    x0T = np.ascontiguousarray(
        np.repeat(np.asarray(embed)[SOS][:, None], BATCH, axis=1)
    ).astype(bf)
    ident = np.eye(P, dtype=np.float32)
    Wo = np.zeros((NCORES * VPAD, HID), np.float32)
    Wo[:VOCAB] = W_out
    bo = np.zeros((NCORES * VPAD,), np.float32)
    bo[:VOCAB] = b_out
    common = dict(
        featT=featT, WhpT=WhpT, WihT=WihT, WhhT=WhhT,
        b_ih=np.asarray(b_ih, np.float32), b_hh=np.asarray(b_hh, np.float32),
        b_hp=np.asarray(b_hp, np.float32), x0T=x0T, ident=ident,
    )
    in_maps = []
    for c in range(NCORES):
        sl = slice(c * VPAD, (c + 1) * VPAD)
        m = dict(common)
        m["WoutT"] = np.ascontiguousarray(Wo[sl].T).astype(bf)
        m["b_out"] = bo[sl].copy()
        in_maps.append(m)
    return in_maps


def kernel(**inputs):
    global LAST_RESULTS
    args = {k: np.asarray(v) for k, v in inputs.items()}
    in_maps = _shard_inputs(
        args["feat"], args["W_hp"], args["b_hp"], args["W_ih"], args["W_hh"],
        args["b_ih"], args["b_hh"], args["embed"], args["W_out"], args["b_out"],
    )
    nc = build()
    res = run_bass_kernel_spmd(nc, in_maps, core_ids=list(range(NCORES)))
    LAST_RESULTS = res
    per_core = []
    for r in res.results:
        blocks = [r[f"OUT{j}"] for j in range(NBLK)]   # each [VPAD, B, BS]
        per_core.append(np.concatenate(blocks, axis=2))  # [VPAD, B, T]
    full = np.concatenate(per_core, axis=0)              # [8*VPAD, B, T]
    out = full[:VOCAB].transpose(1, 0, 2)                # [B, V, T]
    return np.ascontiguousarray(out, dtype=np.float32)


# revision 8
# speedup vs baseline: 1.8939x; 1.2907x over previous
"""Trainium2 Bass kernel for the GRU caption model.

h0 = feat @ W_hp.T + b_hp; 200-step GRU whose hidden-proj gate pre-activations
are step-invariant; logits = h_t @ W_out.T + b_out -> [B, V, T].

v2 design (CoreSim cost-model driven):
- Vocab sharded 8 ways; every core runs the (latency-bound) GRU redundantly.
- GRU per-step chain minimized: step-invariant gate constants are preloaded
  into PSUM with a single N=384 fp32r identity matmul, the elementwise chain
  runs on Pool (GPSIMD, no SBUF/PSUM access-latency penalty) with the two
  tanhs on Act.  h_t is written once, in bf16, straight into the resT
  activation buffer that feeds both the next step's matmul and the
  projection.
- Projection in 4 t-blocks of 50 steps, bf16 weights/activations (1 cyc/row
  at any N), bias folded into the PSUM->SBUF copy (split between DVE+Pool).
- Output DMA: one transfer per (vocab-tile, t-block) with 6400-byte
  contiguous runs (full 360 GB/s); per-block DRAM tensors laid out
  [VPAD, B, BS], transposed/stitched on the host.
"""

import numpy as np
import ml_dtypes

import concourse.bass as bass
import concourse.mybir as mybir
import concourse.tile as tile
from concourse import bacc
from concourse.bass_utils import run_bass_kernel_spmd

F32 = mybir.dt.float32
F32R = mybir.dt.float32r
BF16 = mybir.dt.bfloat16
AF = mybir.ActivationFunctionType
ALU = mybir.AluOpType

VOCAB = 30522
HID = 512
FEAT = 2048
STEPS = 200
BATCH = 32
SOS = 101
NCORES = 8
P = 128
KO = HID // P          # 4 h-chunks
GM = 3 * HID // P      # 12 gate row-groups (r: 0-3, z: 4-7, n: 8-11)
KF = FEAT // P         # 16 feat chunks
VPAD = 3840            # per-core padded vocab rows = 30 * 128
MT = VPAD // P         # 30 vocab tiles per core
NBLK = 4
BS = STEPS // NBLK     # 50 steps per proj block
PBG = 4                # batch group size in proj (N = PBG*BS = 200)
UNITS_PER_STEP = 2     # proj (m,g) units interleaved per GRU step

LAST_RESULTS = None  # test harness introspection


def build():
    nc = bacc.Bacc("TRN2", target_bir_lowering=False, debug=False)

    featT = nc.dram_tensor("featT", [FEAT, BATCH], F32, kind="ExternalInput")
    WhpT = nc.dram_tensor("WhpT", [FEAT, HID], F32, kind="ExternalInput")
    WihT = nc.dram_tensor("WihT", [HID, 3 * HID], BF16, kind="ExternalInput")
    WhhT = nc.dram_tensor("WhhT", [HID, 3 * HID], F32, kind="ExternalInput")
    b_ih = nc.dram_tensor("b_ih", [3 * HID], F32, kind="ExternalInput")
    b_hh = nc.dram_tensor("b_hh", [3 * HID], F32, kind="ExternalInput")
    b_hp = nc.dram_tensor("b_hp", [HID], F32, kind="ExternalInput")
    x0T = nc.dram_tensor("x0T", [HID, BATCH], BF16, kind="ExternalInput")
    ident = nc.dram_tensor("ident", [P, P], F32, kind="ExternalInput")
    WoutT = nc.dram_tensor("WoutT", [HID, VPAD], BF16, kind="ExternalInput")
    b_out = nc.dram_tensor("b_out", [VPAD], F32, kind="ExternalInput")
    OUTS = [
        nc.dram_tensor(f"OUT{j}", [VPAD, BATCH, BS], F32, kind="ExternalOutput")
        for j in range(NBLK)
    ]

    with tile.TileContext(nc) as tc:
        with (
            tc.tile_pool(name="const", bufs=1) as const,
            tc.tile_pool(name="sp", bufs=4) as sp,
            tc.tile_pool(name="stg", bufs=3) as stg,
            tc.tile_pool(name="psg", bufs=2, space="PSUM") as psg,
            tc.tile_pool(name="psp", bufs=4, space="PSUM") as psp,
        ):
            # ---- constants into SBUF (few, large DMAs) ----
            featT_sb = const.tile([P, KF, BATCH], F32, tag="featsb")
            nc.sync.dma_start(featT_sb[:], featT.rearrange("(k p) b -> p k b", p=P))
            whp_sb = const.tile([P, KF, HID], F32, tag="whp")
            nc.sync.dma_start(whp_sb[:], WhpT.rearrange("(k p) h -> p k h", p=P))
            bhp_sb = const.tile([P, KO], F32, tag="bhp")
            nc.sync.dma_start(bhp_sb[:], b_hp.rearrange("(m p) -> p m", p=P))
            bih_sb = const.tile([P, GM], F32, tag="bih")
            nc.sync.dma_start(bih_sb[:], b_ih.rearrange("(m p) -> p m", p=P))
            bhh_sb = const.tile([P, GM], F32, tag="bhh")
            nc.sync.dma_start(bhh_sb[:], b_hh.rearrange("(m p) -> p m", p=P))
            id_sb = const.tile([P, P], F32, tag="id")
            nc.sync.dma_start(id_sb[:], ident[:, :])
            x0_sb = const.tile([P, KO, BATCH], BF16, tag="x0")
            nc.sync.dma_start(x0_sb[:], x0T.rearrange("(k p) b -> p k b", p=P))
            whh_sb = const.tile([P, KO, 3 * HID], F32, tag="whh")
            nc.sync.dma_start(whh_sb[:], WhhT.rearrange("(k p) g -> p k g", p=P))
            wih = const.tile([P, KO, GM, P], BF16, tag="wih")
            nc.sync.dma_start(
                wih[:], WihT.rearrange("(k p) (m c) -> p k m c", p=P, c=P)
            )
            wout_sb = const.tile([P, KO, VPAD], BF16, tag="wout")
            nc.sync.dma_start(
                wout_sb[:], WoutT.rearrange("(k p) v -> p k v", p=P)
            )
            bout_sb = const.tile([P, MT], F32, tag="bout")
            nc.sync.dma_start(bout_sb[:], b_out.rearrange("(m p) -> p m", p=P))

            # ---- h0 = feat @ W_hp.T + b_hp (fp32, exact) ----
            ps_h = psg.tile([P, GM, BATCH], F32, tag="gates", name="psh")
            for ko in range(KO):
                for kf in range(KF):
                    nc.tensor.matmul(
                        ps_h[:, ko, :],
                        whp_sb[:, kf, ko * P:(ko + 1) * P],
                        featT_sb[:, kf, :],
                        start=(kf == 0), stop=(kf == KF - 1),
                    )
            h0T = const.tile([P, KO, BATCH], F32, tag="h0T")
            for ko in range(KO):
                nc.scalar.activation(
                    h0T[:, ko, :], ps_h[:, ko, :], AF.Identity,
                    bias=bhp_sb[:, ko, None], scale=1.0,
                )
            h0_half = const.tile([P, KO, BATCH], F32, tag="h0h")
            nc.scalar.mul(h0_half[:], h0T[:], 0.5)

            # ---- gh = h0 @ W_hh.T + b_hh (fp32, exact; step-invariant) ----
            ps_g = psg.tile([P, GM, BATCH], F32, tag="gates", name="psg2")
            for m in range(GM):
                for k in range(KO):
                    nc.tensor.matmul(
                        ps_g[:, m, :],
                        whh_sb[:, k, m * P:(m + 1) * P],
                        h0T[:, k, :],
                        start=(k == 0), stop=(k == KO - 1),
                    )
            ghT = const.tile([P, GM, BATCH], F32, tag="ghT")
            for m in range(GM):
                nc.scalar.activation(
                    ghT[:, m, :], ps_g[:, m, :], AF.Identity,
                    bias=bhh_sb[:, m, None], scale=1.0,
                )
            # hn2 = 0.5*gh_n (used every step by the r-gating of n)
            hn2 = const.tile([P, KO, BATCH], F32, tag="hn2")
            nc.scalar.mul(hn2[:], ghT[:, 8:12, :], 0.5)
            # C: per-step PSUM preload. rz: gh_rz + b_ih_rz ; n: hn2 + b_ih_n
            C = const.tile([P, GM, BATCH], F32, tag="C")
            nc.vector.tensor_add(
                C[:, 0:8, :], ghT[:, 0:8, :],
                bih_sb[:, 0:8, None].to_broadcast((P, 8, BATCH)),
            )
            nc.vector.tensor_add(
                C[:, 8:12, :], hn2[:],
                bih_sb[:, 8:12, None].to_broadcast((P, KO, BATCH)),
            )

            # resT blocks: h_t in bf16, feeds next-step matmul AND projection
            resT = []
            for j in range(NBLK):
                rt = const.tile(
                    [P, KO, BATCH, BS], BF16, tag=f"resT{j}", name=f"resT{j}"
                )
                resT.append(rt)

            id_r = id_sb.bitcast(F32R)
            C_r = C.bitcast(F32R)

            NG = BATCH // PBG  # 8 batch groups per vocab tile
            proj_fifo = []     # pending (j, m, g) units
            stage_cur = [None]

            def emit_unit():
                j, m, g = proj_fifo.pop(0)
                if g == 0:
                    stage_cur[0] = stg.tile(
                        [P, BATCH, BS], F32, tag="stage", name="stage"
                    )
                stage = stage_cur[0]
                pp = psp.tile([P, PBG, BS], F32, tag="pp", name="pp")
                for k in range(KO):
                    nc.tensor.matmul(
                        pp[:, :, :],
                        wout_sb[:, k, m * P:(m + 1) * P],
                        resT[j][:, k, PBG * g:PBG * g + PBG, :],
                        start=(k == 0), stop=(k == KO - 1),
                    )
                dst = stage[:, PBG * g:PBG * g + PBG, :]
                if g % 2 == 0:
                    nc.gpsimd.tensor_scalar_add(dst, pp[:, :, :], bout_sb[:, m, None])
                else:
                    nc.vector.tensor_scalar_add(dst, pp[:, :, :], bout_sb[:, m, None])
                if g == NG - 1:
                    nc.sync.dma_start(OUTS[j][m * P:(m + 1) * P, :, :], stage[:])

            # ---- GRU steps ----
            for t in range(STEPS):
                j, o = divmod(t, BS)
                ps = psg.tile([P, GM, BATCH], F32, tag="gates", name="ps")
                # preload step-invariant gate constants into all 12 groups
                nc.tensor.matmul(
                    ps[:, :, :], id_r, C_r, start=True, stop=False,
                    skip_group_check=True,
                )
                if t == 0:
                    prev = x0_sb
                else:
                    jp, op = divmod(t - 1, BS)
                    prev = resT[jp][:, :, :, op]
                # gate matmuls: r groups first so the r-tanh fires earliest
                for m in range(GM):
                    for k in range(KO):
                        nc.tensor.matmul(
                            ps[:, m, :], wih[:, k, m, :], prev[:, k, :],
                            start=False, stop=(k == KO - 1),
                            skip_group_check=True,
                        )
                # interleave pending projection work into the step's PE slack
                for _ in range(UNITS_PER_STEP):
                    if proj_fifo:
                        emit_unit()
                # r/z: tanh(0.5*(gi+gh+b)) ; r = (1+t_r)/2, z = (1+t_z)/2
                t_r = sp.tile([P, KO, BATCH], F32, tag="tr")
                nc.scalar.activation(t_r[:], ps[:, 0:4, :], AF.Tanh, scale=0.5)
                t_z = sp.tile([P, KO, BATCH], F32, tag="tz")
                nc.scalar.activation(t_z[:], ps[:, 4:8, :], AF.Tanh, scale=0.5)
                # n = tanh(gi_n + b_ih_n + r*gh_n) with r*gh_n = hn2 + t_r*hn2
                a = sp.tile([P, KO, BATCH], F32, tag="a")
                nc.gpsimd.tensor_mul(a[:], t_r[:], hn2[:])
                sn = sp.tile([P, KO, BATCH], F32, tag="sn")
                nc.gpsimd.tensor_add(sn[:], ps[:, 8:12, :], a[:])
                n = sp.tile([P, KO, BATCH], F32, tag="n")
                nc.scalar.activation(n[:], sn[:], AF.Tanh, scale=1.0)
                # h = (1-z)*n + z*h0 = [0.5n + 0.5h0] + 0.5*t_z*(h0 - n)
                q = sp.tile([P, KO, BATCH], F32, tag="q")
                nc.gpsimd.tensor_sub(q[:], h0T[:], n[:])
                w2 = sp.tile([P, KO, BATCH], F32, tag="w2")
                nc.gpsimd.scalar_tensor_tensor(
                    w2[:], t_z[:], 0.5, q[:], ALU.mult, ALU.mult
                )
                p2 = sp.tile([P, KO, BATCH], F32, tag="p2")
                nc.vector.scalar_tensor_tensor(
                    p2[:], n[:], 0.5, h0_half[:], ALU.mult, ALU.add
                )
                nc.gpsimd.tensor_add(resT[j][:, :, :, o], w2[:], p2[:])
                if o == BS - 1:
                    proj_fifo.extend(
                        (j, m, g) for m in range(MT) for g in range(NG)
                    )
            while proj_fifo:
                emit_unit()

    nc.compile()
    return nc


def _shard_inputs(feat, W_hp, b_hp, W_ih, W_hh, b_ih, b_hh, embed, W_out, b_out):
    bf = ml_dtypes.bfloat16
    featT = np.ascontiguousarray(feat.T, dtype=np.float32)
    WhpT = np.ascontiguousarray(W_hp.T, dtype=np.float32)
    WihT = np.ascontiguousarray(W_ih.T).astype(bf)
    WhhT = np.ascontiguousarray(W_hh.T, dtype=np.float32)
    x0T = np.ascontiguousarray(
        np.repeat(np.asarray(embed)[SOS][:, None], BATCH, axis=1)
    ).astype(bf)
    ident = np.eye(P, dtype=np.float32)
    Wo = np.zeros((NCORES * VPAD, HID), np.float32)
    Wo[:VOCAB] = W_out
    bo = np.zeros((NCORES * VPAD,), np.float32)
    bo[:VOCAB] = b_out
    common = dict(
        featT=featT, WhpT=WhpT, WihT=WihT, WhhT=WhhT,
        b_ih=np.asarray(b_ih, np.float32), b_hh=np.asarray(b_hh, np.float32),
        b_hp=np.asarray(b_hp, np.float32), x0T=x0T, ident=ident,
    )
    in_maps = []
    for c in range(NCORES):
        sl = slice(c * VPAD, (c + 1) * VPAD)
        m = dict(common)
        m["WoutT"] = np.ascontiguousarray(Wo[sl].T).astype(bf)
        m["b_out"] = bo[sl].copy()
        in_maps.append(m)
    return in_maps


def kernel(**inputs):
    global LAST_RESULTS
    args = {k: np.asarray(v) for k, v in inputs.items()}
    in_maps = _shard_inputs(
        args["feat"], args["W_hp"], args["b_hp"], args["W_ih"], args["W_hh"],
        args["b_ih"], args["b_hh"], args["embed"], args["W_out"], args["b_out"],
    )
    nc = build()
    res = run_bass_kernel_spmd(nc, in_maps, core_ids=list(range(NCORES)))
    LAST_RESULTS = res
    per_core = []
    for r in res.results:
        blocks = [r[f"OUT{j}"] for j in range(NBLK)]   # each [VPAD, B, BS]
        per_core.append(np.concatenate(blocks, axis=2))  # [VPAD, B, T]
    full = np.concatenate(per_core, axis=0)              # [8*VPAD, B, T]
    out = full[:VOCAB].transpose(1, 0, 2)                # [B, V, T]
    return np.ascontiguousarray(out, dtype=np.float32)
